# revision 3
# baseline (speedup 1.0000x reference)
"""AdaConv Trainium2 kernel, v8: fp16 I/O, engine rebalance, packed side buffers.

Same math and tiling as v3 (main A tiles [128, 3456]; the 640-col row
tails displaced off slow DMA-engine 15's partitions), but the host packs
the displaced tails into contiguous DRAM side buffers:
  xs/os [120, 8*640]: xs[p, 640t+c] = x[128t+p, 3456+c]   (fast rows' tails)
  xc/oc [64, 640]:    xc[8t+i, c]   = x[128t+120+i, 3456+c] (slow rows' tails)
so the S transfer moves with 120 x 10.2 KB descriptors instead of 1920 x
1.25 KB ones. Small packets cost a near-constant SDMA slot (~200 ns vs
~120 ns for 8 KB), so v3's S stream diluted the mid-phase and drained
descriptor-limited at the end (~5 us tail).
"""

from contextlib import ExitStack

import numpy as np

import concourse.bass as bass
import concourse.tile as tile
from concourse import bacc, mybir
from concourse.bass_utils import run_bass_kernel_spmd

B, C, H, W = 32, 256, 64, 64
N_CORES = 8
BL = B // N_CORES            # 4 samples per core
HWF = H * W                  # 4096
ROWS = BL * C                # 1024 rows (b, c) per core
P = 128
NCH = C // P                 # 2
NT = ROWS // P               # 8 main tiles
F32 = mybir.dt.float32
F16 = mybir.dt.float16

MOVE = 640                   # tail cols displaced off partitions 120-127
KEEP = HWF - MOVE            # 3456
PSLOW = 120
NSLOW = P - PSLOW

W_OFF = {"w1": 0, "w2": 512, "bw1": 1024, "bw2": 1536}
B_OFF = {"b1": 2048, "b2": 2050, "bb1": 2052, "bb2": 2054}
L_OFF = 2056
PACK_COLS = L_OFF + NCH * BL  # 2064

_COMPILED_NC = None


def _mlp_branch(tc, pool, psum, wp, wkey1, bkey1, wkey2, bkey2, s8b, xoff,
                name):
    nc = tc.nc
    o1, o2 = W_OFF[wkey1], W_OFF[wkey2]
    h1T = []
    for hj in range(NCH):
        ps = psum.tile([P, BL], F32, tag="ps_mm")
        for ci in range(NCH):
            nc.tensor.matmul(
                ps[:],
                wp[:, o1 + ci * C + hj * P: o1 + ci * C + (hj + 1) * P],
                wp[:, L_OFF + ci * BL: L_OFF + (ci + 1) * BL],
                start=(ci == 0), stop=(ci == NCH - 1),
            )
        h = pool.tile([P, BL], F32, tag=f"{name}_h{hj}")
        nc.vector.tensor_scalar(
            h[:], ps[:], wp[:, B_OFF[bkey1] + hj: B_OFF[bkey1] + hj + 1], 0.0,
            mybir.AluOpType.add, mybir.AluOpType.max,
        )
        h1T.append(h)
    for oj in range(NCH):
        ps = psum.tile([P, BL], F32, tag="ps_mm")
        for hi in range(NCH):
            nc.tensor.matmul(
                ps[:],
                wp[:, o2 + hi * C + oj * P: o2 + hi * C + (oj + 1) * P],
                h1T[hi][:],
                start=(hi == 0), stop=(hi == NCH - 1),
            )
        dst = s8b[:, 8 * xoff + oj: 8 * xoff + 8: 2]
        nc.vector.tensor_scalar(
            dst, ps[:], wp[:, B_OFF[bkey2] + oj: B_OFF[bkey2] + oj + 1], None,
            mybir.AluOpType.add,
        )


def _build_body(ctx, tc, aps):
    nc = tc.nc
    xa, xs, xc = aps["xa"], aps["xs"], aps["xc"]
    oa, os_, oc = aps["oa"], aps["os"], aps["oc"]

    const = ctx.enter_context(tc.tile_pool(name="const", bufs=1))
    mlp_pool = ctx.enter_context(tc.tile_pool(name="mlp", bufs=1))
    psum = ctx.enter_context(tc.tile_pool(name="psum", bufs=2, space="PSUM"))

    wp = const.tile([P, PACK_COLS], F32)
    nc.scalar.dma_start(wp[:], aps["wpack"][:, :])

    # bulk loads first, split across both HWDGE rings
    xpool = ctx.enter_context(tc.tile_pool(name="x", bufs=NT))
    side = ctx.enter_context(tc.tile_pool(name="side", bufs=1))
    ats = []
    for t in range(NT):
        at = xpool.tile([P, KEEP], F16)
        nc.scalar.dma_start(at[:], xa[t * P:(t + 1) * P, :])
        ats.append(at)
    st = side.tile([PSLOW, NT * MOVE], F16)
    ct = side.tile([NSLOW * NT, MOVE], F16)
    nc.scalar.dma_start(st[:], xs[:, :])
    nc.gpsimd.dma_start(ct[:], xc[:, :])

    # s8b[:, t] = scale column for tile t; s8b[:, 8 + t] = bias column
    s8b = mlp_pool.tile([P, 16], F32)
    _mlp_branch(tc, mlp_pool, psum, wp, "w1", "b1", "w2", "b2", s8b, 0, "sc")
    _mlp_branch(tc, mlp_pool, psum, wp, "bw1", "bb1", "bw2", "bb2", s8b, 1, "bi")

    # gathered scalars for C: csc[8t+i, :] = s8b[120+i, (t, 8+t)]
    csc = side.tile([NSLOW * NT, 2], F32)
    for t in range(NT):
        nc.gpsimd.dma_start(
            csc[t * NSLOW:(t + 1) * NSLOW, :],
            s8b[PSLOW:P, t:t + NT + 1:NT],
        )

    for t in range(NT):
        at = ats[t]
        nc.vector.tensor_scalar(
            at[:], at[:], s8b[:, t:t + 1], s8b[:, 8 + t:8 + t + 1],
            mybir.AluOpType.mult, mybir.AluOpType.add,
        )
        nc.sync.dma_start(oa[t * P:(t + 1) * P, :], at[:])
        nc.vector.tensor_scalar(
            st[:, t * MOVE:(t + 1) * MOVE], st[:, t * MOVE:(t + 1) * MOVE],
            s8b[0:PSLOW, t:t + 1], s8b[0:PSLOW, 8 + t:8 + t + 1],
            mybir.AluOpType.mult, mybir.AluOpType.add,
        )
    nc.vector.tensor_scalar(
        ct[:], ct[:], csc[:, 0:1], csc[:, 1:2],
        mybir.AluOpType.mult, mybir.AluOpType.add,
    )
    nc.sync.dma_start(os_[:, :], st[:])
    nc.gpsimd.dma_start(oc[:, :], ct[:])


def build_nc():
    nc = bacc.Bacc("TRN2", debug=False, num_devices=N_CORES)
    dp = nc.declare_dram_parameter
    aps = {
        "xa": dp("xa", [ROWS, KEEP], F16, isOutput=False).ap(),
        "xs": dp("xs", [PSLOW, NT * MOVE], F16, isOutput=False).ap(),
        "xc": dp("xc", [NSLOW * NT, MOVE], F16, isOutput=False).ap(),
        "wpack": dp("wpack", [P, PACK_COLS], F32, isOutput=False).ap(),
        "oa": dp("oa", [ROWS, KEEP], F16, isOutput=True).ap(),
        "os": dp("os", [PSLOW, NT * MOVE], F16, isOutput=True).ap(),
        "oc": dp("oc", [NSLOW * NT, MOVE], F16, isOutput=True).ap(),
    }
    with tile.TileContext(nc) as tc, ExitStack() as ctx:
        _build_body(ctx, tc, aps)
    nc.compile()
    return nc


def _get_nc():
    global _COMPILED_NC
    if _COMPILED_NC is None:
        _COMPILED_NC = build_nc()
    return _COMPILED_NC


def _make_wpack(inputs, core):
    wp = np.empty((P, PACK_COLS), dtype=np.float32)
    for k in ("w1", "w2", "bw1", "bw2"):
        wT = np.asarray(inputs[k], dtype=np.float32).T
        o = W_OFF[k]
        for ci in range(NCH):
            wp[:, o + ci * C: o + (ci + 1) * C] = wT[ci * P:(ci + 1) * P, :]
    for k in ("b1", "b2", "bb1", "bb2"):
        bcol = np.asarray(inputs[k], dtype=np.float32).reshape(NCH, P).T
        wp[:, B_OFF[k]: B_OFF[k] + NCH] = bcol
    lat = np.asarray(inputs["latent"], dtype=np.float32).reshape(B, C)
    lT = lat[core * BL:(core + 1) * BL, :].T
    for ci in range(NCH):
        wp[:, L_OFF + ci * BL: L_OFF + (ci + 1) * BL] = lT[ci * P:(ci + 1) * P, :]
    return wp


def make_in_maps(inputs):
    x16 = np.asarray(inputs["x"]).astype(np.float16)
    in_maps = []
    for i in range(N_CORES):
        x3 = x16[i * BL:(i + 1) * BL].reshape(NT, P, HWF)
        in_maps.append({
            "xa": np.ascontiguousarray(x3[:, :, :KEEP]).reshape(ROWS, KEEP),
            "xs": np.ascontiguousarray(
                x3[:, :PSLOW, KEEP:].transpose(1, 0, 2)
            ).reshape(PSLOW, NT * MOVE),
            "xc": np.ascontiguousarray(x3[:, PSLOW:, KEEP:]).reshape(
                NSLOW * NT, MOVE
            ),
            "wpack": _make_wpack(inputs, i),
        })
    return in_maps


def run(inputs, trace=False, **kwargs):
    nc = _get_nc()
    in_maps = make_in_maps(inputs)
    res = run_bass_kernel_spmd(
        nc, in_maps, core_ids=list(range(N_CORES)), trace=trace, **kwargs
    )
    shards = []
    for i in range(N_CORES):
        r = res.results[i]
        o3 = np.empty((NT, P, HWF), dtype=np.float32)
        o3[:, :, :KEEP] = np.asarray(r["oa"]).astype(np.float32).reshape(NT, P, KEEP)
        o3[:, :PSLOW, KEEP:] = (
            np.asarray(r["os"]).astype(np.float32)
            .reshape(PSLOW, NT, MOVE).transpose(1, 0, 2)
        )
        o3[:, PSLOW:, KEEP:] = (
            np.asarray(r["oc"]).astype(np.float32).reshape(NT, NSLOW, MOVE)
        )
        shards.append(o3.reshape(BL, C, H, W))
    return np.concatenate(shards, axis=0), res


def kernel(**inputs):
    out, _ = run(inputs, trace=False)
    return out
